# revision 47
# baseline (speedup 1.0000x reference)
"""Multi-head self-attention on 8 TRN2 NeuronCores.

Problem: x[2,2048,1024] -> qkv proj -> 16-head attention -> out proj.
Sharding: core c handles batch b=c//4 and head group g=c%4 (4 heads each).
Each core computes a partial output y_c[2048,1024] = attn_out_heads(g) @ W_proj[rows g];
host sums the 4 partials per batch and adds b_proj.

Schedule (v4, ping-pong): the attention stream is ACT(exp)-bound at
~578ns per [128,512] exp tile (256 tiles ~= 148us busy) and the PE's
total matmul work is ~148us, so the two must overlap near-perfectly.
The repeat loop body holds TWO sub-iterations with double-buffered
stage-1 activations (qk/kpad/v4 sets A and B): sub-iteration A reads
set A while pumping set B's qkv-projection blocks, and vice versa.
Stage-1 blocks therefore have NO consumption deadline (used only by the
next sub-iteration) and NO write-after-read hazard (last read a full
sub-iteration ago), so they spread perfectly evenly (one 8-matmul block
every 8 microsteps) and the PE never stalls on them; a one-time dense
prologue seeds set A.  Other structure:
  - attention: 512-wide microsteps (scores -> exp -> PV), 4-deep ss
    PSUM rotation, per-i-range head order (1,3,0,2) so the entries that
    release projection columns are even heads (their normalize needs no
    partition-shift DMA); one [65,512] po PSUM tile per entry.
  - softmax normalize: denominator row shifted to a partition-0 tile,
    reciprocal_approx_fast (~5x faster than InstReciprocal), gpsimd
    partition_broadcast, multiply straight out of PSUM.
  - out-projection: 16 steps of 4 interleaved fp32r matmuls; the 4
    steps depending on the final normalize rotate into the NEXT
    sub-iteration's first microsteps (outT values are idempotent across
    iterations), so the tail never stalls the PE queue.
  - layouts: x transposed on host, q/k produced transposed (qkT[f,s]),
    k zero-padded to K=128, v natural with a ones column per head so
    the PV matmul also yields softmax denominators.
  - bf16 for x, W1, q, k, v, exp(probs) and the y partials; fp32r for
    outT/W_proj so the final projection stays accurate.

Measured (neuron-profile traces, steady-state repeat loop): ~173-177us
per iteration per core vs 252us for the serial-stage-1 baseline; PE
~95% busy at its sustained ~2.37GHz (216ns per N=512 matmul), ACT ~91%
busy.  Output rel-inf error ~6.8e-3 vs f64 (gate 2e-2).  The stage-1
pump is emitted in QUARTER bursts (~0.65us) across jc3-6: a contiguous
2.6us pump burst delays the next scores matmul beyond what the 4-deep
ss buffer can absorb and starves ACT (whole-pump measured +5us/iter,
half-pump +2).  Known residual costs: ~4us soft-barrier at the For_i
back-edge (staggered-reset decs wait on the final normalize's DVE
chain + a conservative per-pass ACT_TABLE_LOAD) and PE p-state
inflation on exp-dependent PV matmuls at entry boundaries.  Tried and
rejected: [128,1024] exp tiles (ACT is ~1.15ns/elem regardless, and
coarser granularity added PV stalls), For_i unroll=4 (instruction-
fetch pressure outweighed the halved seam), fp8 DoubleRow scores
(precision margin too thin), pump@jc2 + proj@jc11 (+35us/iter from
shifted ss emission), deferring the final normalize across the back
edge via a persistent SBUF snapshot (net-neutral: congests the
pass-start DVE queue by what it saves at the barrier), and
staggered_reset=False (second pass identical, first pass worse).
Do NOT re-copy loop-invariant constants (v4 ones columns) inside the
loop: the scheduler hoists dep-free copies to the head of each pass's
DVE queue and delays the rotated projections' PSUM evacuations ~3us.
The steady-state stream shows ZERO matmuls at the 192ns burst clock —
2.37GHz is the sustained PE rate here and is not schedule-addressable.
"""

import numpy as np

N_CORES = 8
B, S, D = 2, 2048, 1024
H, HD = 16, 64
HPC = 4          # heads per core
F_QK = 512      # q+k features per core (4 heads x 64 x 2)
F_V = 256       # v features per core
FT = 768        # total qkv features per core
SC = 512        # seq chunk (matmul N)
NSC = S // SC   # 4
NJ = S // 128   # 16 j-blocks
NDC = D // 128  # 8 contraction chunks

_CACHE = {}


def _build(repeat=1):
    import contextlib
    import concourse.bass as bass  # noqa: F401
    import concourse.mybir as mybir
    import concourse.tile as tile
    from concourse import bacc

    F32, F32R = mybir.dt.float32, mybir.dt.float32r
    BF16 = mybir.dt.bfloat16

    nc = bacc.Bacc("TRN2", target_bir_lowering=False, num_devices=N_CORES)
    xT = nc.declare_dram_parameter("xT", [D, S], BF16, isOutput=False)
    W1 = nc.declare_dram_parameter("W1", [D, FT], BF16, isOutput=False)
    b1 = nc.declare_dram_parameter("b1", [FT, 1], F32, isOutput=False)
    Wp = nc.declare_dram_parameter("Wp", [HPC * HD, D], F32R, isOutput=False)
    y = nc.declare_dram_parameter("y", [S, D], BF16, isOutput=True)

    with tile.TileContext(nc) as tc:
        with (
            tc.tile_pool(name="weights", bufs=1) as wpool,
            tc.tile_pool(name="persist", bufs=1) as persist,
            tc.tile_pool(name="xin", bufs=1) as xpool,
            tc.tile_pool(name="etile", bufs=12) as epool,
            tc.tile_pool(name="yout", bufs=4) as ypool,
            tc.tile_pool(name="small", bufs=4) as spool,
            tc.tile_pool(name="psA", bufs=2, space="PSUM") as psA,
            tc.tile_pool(name="psS", bufs=2, space="PSUM") as psS,
            tc.tile_pool(name="psO", bufs=2, space="PSUM") as psO,
        ):
            # ---- inputs; chunks of 4 dc-rows go out as single
            # multi-dim-AP DMAs (each InstDMACopy splits across all 16 SDMA
            # engines; dispatch is ~625ns each, so few + big wins).
            def rows_dma(dst, dst_w, src, r0, nr, src_c0, ncol, dst_c0=None):
                """dst[:, j*dst_w+dst_c0 :+ncol] <-
                src[(r0+j)*128:(r0+j+1)*128, src_c0:src_c0+ncol] per j"""
                if dst_c0 is None:
                    dst_c0 = src_c0
                s = src[r0 * 128:r0 * 128 + 1, 0:1]
                width = src.shape[-1]
                in_ap = bass.AP(tensor=s.tensor, offset=s.offset + src_c0,
                                ap=[[width, 128], [128 * width, nr], [1, ncol]])
                pp = dst.ap[0][0]
                out_ap = bass.AP(tensor=dst.tensor, offset=dst.offset + dst_c0,
                                 ap=[[pp, 128], [dst_w, nr], [1, ncol]])
                nc.sync.dma_start(out=out_ap, in_=in_ap)

            w1b = [wpool.tile([128, 4 * FT], BF16, tag=f"w1b_{g}",
                              name=f"w1b_{g}") for g in range(2)]
            w1t = [w1b[dc // 4][:, (dc % 4) * FT:(dc % 4 + 1) * FT]
                   for dc in range(NDC)]
            xb = [[xpool.tile([128, 4096], BF16, tag=f"xb_{pair}_{g}",
                              name=f"xb_{pair}_{g}") for g in range(2)]
                  for pair in range(2)]
            xts2 = [[xb[pair][dc // 4][:, (dc % 4) * 1024:(dc % 4 + 1) * 1024]
                     for dc in range(NDC)] for pair in range(2)]
            rows_dma(w1b[0], FT, W1, 0, 4, 0, 384)
            rows_dma(xb[0][0], 1024, xT, 0, 4, 0, 512)
            rows_dma(w1b[1], FT, W1, 4, 4, 0, 384)
            rows_dma(xb[0][1], 1024, xT, 4, 4, 0, 512)
            rows_dma(w1b[0], FT, W1, 0, 4, 384, 384)
            rows_dma(w1b[1], FT, W1, 4, 4, 384, 384)
            rows_dma(xb[0][0], 1024, xT, 0, 4, 512, 512)
            rows_dma(xb[0][1], 1024, xT, 4, 4, 512, 512)
            b6 = wpool.tile([128, 6], F32, tag="b6", name="b6")
            b1s = b1[0:128, 0:1]
            b6_ap = bass.AP(tensor=b1s.tensor, offset=b1s.offset,
                            ap=[[1, 128], [128, 6]])
            nc.sync.dma_start(out=b6, in_=b6_ap)
            bv = wpool.tile([128, F_V], F32, tag="bv", name="bv")
            bvsrc = b1[F_QK:FT, 0:1]
            bv_ap = bass.AP(tensor=bvsrc.tensor, offset=bvsrc.offset,
                            ap=[[0, 128], [1, F_V]])
            nc.sync.dma_start(out=bv, in_=bv_ap)
            ones = wpool.tile([128, 1], F32, tag="ones", name="ones")
            nc.vector.memset(ones, 1.0)
            rows_dma(xb[1][0], 1024, xT, 0, 4, 1024, 1024, dst_c0=0)
            rows_dma(xb[1][1], 1024, xT, 4, 4, 1024, 1024, dst_c0=0)
            wpt = []
            for p in range(2):
                t = wpool.tile([128, D], F32R, tag=f"wp_{p}", name=f"wp_{p}")
                nc.sync.dma_start(out=t, in_=Wp[p * 128:(p + 1) * 128, :])
                wpt.append(t)

            # ---- double-buffered stage-1 activation sets + shared outT.
            # Constant parts (k zero-pad halves, v ones columns) are written
            # once here; per-iteration writes only touch the data parts.
            nsets = 1 if repeat == 1 else 2
            sets = []
            for si in range(nsets):
                qk = [persist.tile([128, S], BF16, tag=f"qk{si}_{p}",
                                   name=f"qk{si}_{p}") for p in range(2)]
                kpad = [wpool.tile([128, S], BF16, tag=f"kpad{si}_{h}",
                                   name=f"kpad{si}_{h}") for h in range(HPC)]
                for h in range(HPC):
                    zr = slice(64, 128) if h % 2 == 0 else slice(0, 64)
                    nc.vector.memset(kpad[h].bitcast(F32)[zr, :], 0.0)
                v4 = [wpool.tile([128, HPC * (HD + 1)], BF16,
                                 tag=f"v4{si}_{jc}", name=f"v4{si}_{jc}")
                      for jc in range(NJ)]
                for jc in range(NJ):
                    for h in range(HPC):
                        nc.vector.tensor_copy(
                            v4[jc][:, h * (HD + 1) + HD:(h + 1) * (HD + 1)],
                            ones)
                sets.append({"qk": qk, "kpad": kpad, "v4": v4})
            outT = [persist.tile([128, S], F32R, tag=f"outT_{p}",
                                 name=f"outT_{p}") for p in range(2)]
            for p in range(2):
                nc.vector.memset(outT[p].bitcast(F32), 0.0)

            env = dict(w1t=w1t, xts2=xts2, b6=b6, bv=bv, ones=ones, wpt=wpt,
                       y=y, sets=sets, outT=outT, epool=epool, ypool=ypool,
                       spool=spool, psA=psA, psS=psS, psO=psO)

            if repeat == 1:
                # single-shot: stage-1 self-pumps inside the stream
                _emit_stream(nc, mybir, env, 0, None, rotate=False)
            else:
                # prologue: dense stage-1 for set 0
                _emit_stage1_dense(nc, mybir, env, 0)
                # one ping-pong pair per loop pass (unroll=4 measured
                # slower: bigger body loses more to instruction fetch than
                # the halved back-edge seam saves)
                unroll = 2
                assert repeat % unroll == 0, "repeat must be even"
                ET = mybir.EngineType
                with tc.For_i(0, repeat // unroll, 1,
                              hint_engines=(ET.PE, ET.DVE, ET.Activation,
                                            ET.Pool, ET.SP),
                              staggered_reset=True):
                    for _ in range(unroll // 2):
                        _emit_stream(nc, mybir, env, 0, 1, rotate=True)
                        _emit_stream(nc, mybir, env, 1, 0, rotate=True)
    nc.compile()
    return nc


def _stage1_specs(nc, mybir, env, si):
    """Stage-1 block specs (8-matmul accumulation chains) for set `si`."""
    F32 = mybir.dt.float32
    w1t, b6, bv, ones = env["w1t"], env["b6"], env["bv"], env["ones"]
    xts2, psA = env["xts2"], env["psA"]
    qk = env["sets"][si]["qk"]
    kpad = env["sets"][si]["kpad"]
    v4 = env["sets"][si]["v4"]

    def xts(sc, dc):
        return xts2[sc // 2][dc][:, (sc % 2) * SC:(sc % 2 + 1) * SC]

    def qk_spec(sc, fb):
        def alloc():
            return psA.tile([128, SC], F32, tag="mm", name="pq")

        def mm(t, dc):
            nc.tensor.matmul(t, w1t[dc][:, fb * 128:(fb + 1) * 128],
                             xts(sc, dc), start=(dc == 0), stop=(dc == NDC - 1))

        def tail(t):
            ssl1 = slice(sc * SC, (sc + 1) * SC)
            if fb < 2:
                nc.vector.tensor_scalar_add(qk[fb][:, ssl1], t,
                                            b6[:, fb:fb + 1])
            else:
                ke, ko = kpad[2 * (fb - 2)], kpad[2 * (fb - 2) + 1]
                nc.vector.tensor_scalar_add(ke[0:64, ssl1], t[0:64, :],
                                            b6[0:64, fb:fb + 1])
                nc.vector.tensor_scalar_add(ko[64:128, ssl1], t[64:128, :],
                                            b6[64:128, fb:fb + 1])
        return alloc, mm, tail

    def v_spec(sc, sb):
        jc = sc * 4 + sb

        def alloc():
            return psA.tile([128, F_V], F32, tag="mm", name="pv")

        def mm(t, dc):
            nc.tensor.matmul(t, xts(sc, dc)[:, sb * 128:(sb + 1) * 128],
                             w1t[dc][:, F_QK:FT],
                             start=(dc == 0), stop=(dc == NDC - 1))

        def tail(t):
            # NOTE: the ones columns are loop-invariant (written once
            # outside the repeat loop, like kpad's zero halves) — do NOT
            # re-copy them here: the scheduler hoists such dep-free copies
            # to the head of each pass's DVE queue, delaying the rotated
            # projections' PSUM evacuations by ~3us.
            for h in range(HPC):
                nc.vector.tensor_add(
                    v4[jc][:, h * (HD + 1):h * (HD + 1) + HD],
                    t[:, h * HD:(h + 1) * HD], bv[:, h * HD:(h + 1) * HD])
        return alloc, mm, tail

    qks = [qk_spec(sc, fb) for sc in range(NSC) for fb in range(4)]
    vs = [v_spec(sc, sb) for sc in range(NSC) for sb in range(4)]
    return qks, vs


def _make_pump(nc):
    """Block pump: emits 8-matmul blocks with a 4-matmul skew so
    consecutive blocks' accumulation chains interleave (sequential
    accumulating chains measure ~16% slower on HW). A block's data is
    ready only after the NEXT pump call; pump(None) flushes."""
    pending = [None]

    def pump(spec):
        if spec is None:
            if pending[0] is not None:
                mmf, tailf, t = pending[0]
                for dc in range(4, NDC):
                    mmf(t, dc)
                tailf(t)
                pending[0] = None
            return
        alloc, mmf, tailf = spec
        t = alloc()
        for dc in range(4):
            if pending[0] is not None:
                pending[0][0](pending[0][2], 4 + dc)
            mmf(t, dc)
        if pending[0] is not None:
            pending[0][1](pending[0][2])
        pending[0] = (mmf, tailf, t)
    return pump


def _emit_stage1_dense(nc, mybir, env, si):
    pump = _make_pump(nc)
    qks, vs = _stage1_specs(nc, mybir, env, si)
    for qs, vsp in zip(qks, vs):
        pump(qs)
        pump(vsp)
    pump(None)


def _emit_stream(nc, mybir, env, rs, ws, rotate):
    """One sub-iteration: attention + projection reading stage-1 set `rs`,
    while pumping set `ws`'s stage-1 blocks (1 per 8 microsteps; ws=None
    skips). rotate=True defers the 4 final-normalize-dependent projection
    steps to the NEXT sub-iteration's first microsteps (and runs the
    previous one's here) — outT contents are idempotent across
    sub-iterations, so the rotated steps read identical values."""
    from collections import deque

    F32, F32R = mybir.dt.float32, mybir.dt.float32r
    BF16 = mybir.dt.bfloat16
    AF = mybir.ActivationFunctionType
    wpt, y, outT = env["wpt"], env["y"], env["outT"]
    epool, ypool, spool = env["epool"], env["ypool"], env["spool"]
    psA, psS, psO = env["psA"], env["psS"], env["psO"]
    qk = env["sets"][rs]["qk"]
    kpad = env["sets"][rs]["kpad"]
    v4 = env["sets"][rs]["v4"]

    def pump2(sa, sb):
        """Emit two stage-1 blocks fully interleaved (every accumulation
        step has a foreign matmul between its own). Both psA slots are
        released within the call, so projection steps at other microsteps
        never contend with a held slot."""
        aa, ma, ta = sa
        t_a = aa()
        if sb is None:
            for dc in range(NDC):
                ma(t_a, dc)
            ta(t_a)
            return
        ab, mb, tb = sb
        t_b = ab()
        for dc in range(NDC):
            ma(t_a, dc)
            mb(t_b, dc)
        ta(t_a)
        tb(t_b)

    # split-pump: halves the contiguous PE burst (2.6us -> 1.3us) so the
    # 4-deep ss buffer can absorb it without starving ACT
    pend = [None]

    def pump_part(sa=None, sb=None, dc0=0, dc1=4, last=False):
        if sa is not None:
            aa, ma, ta = sa
            ab, mb, tb = sb
            pend[0] = (ma, ta, aa(), mb, tb, ab())
        ma, ta, t_a, mb, tb, t_b = pend[0]
        for dc in range(dc0, dc1):
            ma(t_a, dc)
            mb(t_b, dc)
        if last:
            ta(t_a)
            tb(t_b)
            pend[0] = None

    def make_proj_step(sblk, tail=False):
        ssl = slice(sblk * 128, (sblk + 1) * 128)

        def step():
            ysb = ypool.tile([128, 1024], BF16, tag="ysb", name="ysb")
            py0 = psA.tile([128, SC], F32, tag="mm", name="py0")
            py1 = psA.tile([128, SC], F32, tag="mm", name="py1")
            for p in range(2):
                nc.tensor.matmul(py0, outT[p][:, ssl], wpt[p][:, 0:SC],
                                 start=(p == 0), stop=(p == 1))
                nc.tensor.matmul(py1, outT[p][:, ssl], wpt[p][:, SC:1024],
                                 start=(p == 0), stop=(p == 1))
            if tail:
                nc.scalar.copy(ysb[:, 0:SC], py0)
            else:
                nc.vector.tensor_copy(ysb[:, 0:SC], py0)
            nc.vector.tensor_copy(ysb[:, SC:1024], py1)
            nc.sync.dma_start(out=y[ssl, :], in_=ysb)
        return step

    workq = deque()

    def drain(n=1):
        for _ in range(n):
            if workq:
                workq.popleft()()

    def normalize(h, i0, po):
        # chain length matters: the staggered-reset soft barrier at the
        # loop back-edge waits for the LAST entry's normalize, so read the
        # PSUM po tile directly instead of copying it to SBUF first.
        # reciprocal_approx_fast (~5x faster than InstReciprocal) and
        # partition_broadcast both misread single-partition APs at base 64,
        # so shift the denominator row to a base-0 tile first.
        p = h // 2
        isl = slice(i0, i0 + SC)
        den0 = spool.tile([1, SC], F32, tag="den0", name="den0")
        nc.vector.tensor_copy(den0, po[HD:HD + 1, :])
        recip = spool.tile([1, SC], F32, tag="recip", name="recip")
        nc.vector.reciprocal_approx_fast(recip, den0)
        rb = spool.tile([HD, SC], F32, tag="rb", name="rb")
        nc.gpsimd.partition_broadcast(rb, recip)
        if h % 2 == 0:
            nc.vector.tensor_mul(outT[p][0:HD, isl], po[0:HD, :], rb)
        else:
            tmp = spool.tile([HD, SC], F32R, tag="tmp64", name="tmp64")
            nc.vector.tensor_mul(tmp, po[0:HD, :], rb)
            nc.sync.dma_start(out=outT[p][HD:128, isl], in_=tmp)

    # entries: all 512 wide; odd heads first per i-range so the entries
    # that release projection columns are even heads.
    entries = []
    for ir in range(2):
        for h in (1, 3, 0, 2):
            for half in range(2):
                entries.append((h, ir * 1024 + half * SC))

    # Per-(entry, jc) extra-PE-work schedule, balanced so every entry gets
    # ~3.1us of filler (attention alone is 6.1us PE vs 9.25us ACT).
    # rotate=True: ALL 16 projection steps read the PREVIOUS sub-iteration's
    # outT (idempotent values) and are scheduled before this sub-iteration's
    # first overwrite of their columns: cols 0-511 first written at (0,15),
    # 512-1023 at (1,15), 1024-1535 at (8,15), 1536-2047 at (9,15).
    sched = {}
    selfpump = None
    if rotate:
        for i in range(4):
            sched[(0, 2 + 4 * i)] = [("proj", i)]          # sblk 0-3
            sched[(1, 2 + 4 * i)] = [("proj", 4 + i)]      # sblk 4-7
        if ws is not None:
            # pump placement (jc3) and slot-consumer placement (jc10) are
            # empirically tuned: jc2/jc11/jc13 variants measured 35us/iter
            # SLOWER (ACT starvation from the shifted ss emission).
            qks, vs = _stage1_specs(nc, mybir, env, ws)

            def qsched(e, a, b):
                sched[(e, 3)] = [("pumpA", a, b)]
                sched[(e, 4)] = [("pumpQ", 2, 4, False)]
                sched[(e, 5)] = [("pumpQ", 4, 6, False)]
                sched[(e, 6)] = [("pumpQ", 6, 8, True)]

            for i in range(8):                             # e2-e9
                qsched(2 + i, qks[i], vs[i])
                sched[(2 + i, 10)] = [("proj", 8 + i)]     # sblk 8-15
            qsched(10, qks[8], qks[9])
            qsched(11, qks[10], qks[11])
            for j in range(4):                             # e12-e15
                qsched(12 + j, qks[12 + j], vs[8 + j])
                sched[(12 + j, 10)] = [("pump", vs[12 + j], None)]
    elif ws is None:
        # single-shot path: pump THIS sub-iteration's own stage-1 into the
        # stream, dependency-paced (block X completes, in emission order, at
        # the pump after its own; scores(jc) is emitted one microstep
        # early, PV(jc) within its own microstep).  Lets the exp stream
        # start ~5us in, overlapping the input DMAs and qkv projection.
        qks, vs = _stage1_specs(nc, mybir, env, rs)

        def q(sc, fb):
            return qks[4 * sc + fb]

        def v(sc, sb):
            return vs[4 * sc + sb]

        selfpump = _make_pump(nc)
        for spec in (q(0, 0), q(0, 2), v(0, 0), v(0, 1)):
            selfpump(spec)
        sched = {
            (0, 0): [v(0, 2)], (0, 1): [v(0, 3)],
            (0, 2): [q(1, 2)], (0, 3): [v(1, 0)],
            (0, 4): [v(1, 1)], (0, 5): [v(1, 2)],
            (0, 6): [v(1, 3), q(2, 2)], (0, 7): [v(2, 0)],
            (0, 8): [v(2, 1)], (0, 9): [v(2, 2)],
            (0, 10): [v(2, 3), q(3, 2)], (0, 11): [v(3, 0)],
            (0, 12): [v(3, 1)], (0, 13): [v(3, 2)],
            (0, 14): [v(3, 3), q(1, 0)], (0, 15): [q(0, 3)],
            (1, 0): [q(1, 3)], (1, 2): [q(2, 3)],
            (1, 4): [q(3, 3)], (1, 6): [q(0, 1)],
            (1, 8): [q(1, 1)], (1, 10): [q(2, 0)],
            (1, 12): [q(3, 0)], (1, 14): [q(2, 1)],
            (2, 0): [q(3, 1)], (2, 2): [None],
        }
        sched = {k: [("spump", s) for s in vv] for k, vv in sched.items()}

    msteps = [(e, jc) for e in range(len(entries)) for jc in range(NJ)]

    def make_ss(ent, jc):
        h, i0 = ent
        ss = psS.tile([128, SC], F32, tag="ss", bufs=4, name="ss")
        nc.tensor.matmul(ss, kpad[h][:, jc * 128:(jc + 1) * 128],
                         qk[h // 2][:, i0:i0 + SC], start=True, stop=True)
        return ss

    po_by_e = {}
    sstile = make_ss(entries[0], 0)
    for mi, (e, jc) in enumerate(msteps):
        h, i0 = entries[e]
        for action in sched.get((e, jc), []):
            if action[0] == "pump":
                pump2(action[1], action[2])
            elif action[0] == "pumpA":
                pump_part(action[1], action[2], 0, 2)
            elif action[0] == "pumpQ":
                pump_part(None, None, action[1], action[2], action[3])
            elif action[0] == "spump":
                selfpump(action[1])
            else:
                make_proj_step(action[1])()
        if jc == 0:
            po_by_e[e] = psO.tile([HD + 1, SC], F32, tag="po", name="po")
        ex = epool.tile([128, SC], BF16, tag="ex", bufs=12, name="ex")
        nc.scalar.activation(ex, sstile, AF.Exp, bias=0.0, scale=0.125)
        if mi + 1 < len(msteps):
            ne, njc = msteps[mi + 1]
            sstile = make_ss(entries[ne], njc)
        nc.tensor.matmul(po_by_e[e],
                         v4[jc][:, h * (HD + 1):(h + 1) * (HD + 1)], ex,
                         start=(jc == 0), stop=(jc == NJ - 1))
        if jc == NJ - 1:
            normalize(h, i0, po_by_e.pop(e))
            if not rotate:
                if e == 7:
                    for sblk in range(8):
                        workq.append(make_proj_step(sblk))
                elif e == 14:
                    for sblk in range(8, 12):
                        workq.append(make_proj_step(sblk))
                elif e == 15:
                    for sblk in range(12, 16):
                        workq.append(make_proj_step(sblk, tail=True))
        if not rotate and e >= 8 and jc in (4, 12):
            drain(1)
    while workq:
        workq.popleft()()


def _shards(x, W_qkv, b_qkv, W_proj):
    """Build per-core input maps."""
    import ml_dtypes
    bf16 = ml_dtypes.bfloat16
    xTb = [np.ascontiguousarray(x[b].T.astype(bf16)) for b in range(B)]
    in_maps = []
    for c in range(N_CORES):
        b, g = c // 4, c % 4
        cols = slice(g * HPC * HD, (g + 1) * HPC * HD)  # 256 cols within q/k/v
        W1 = np.concatenate([W_qkv[:, 0 * D:1 * D][:, cols],
                             W_qkv[:, 1 * D:2 * D][:, cols],
                             W_qkv[:, 2 * D:3 * D][:, cols]], axis=1)
        b1 = np.concatenate([b_qkv[0 * D:1 * D][cols],
                             b_qkv[1 * D:2 * D][cols],
                             b_qkv[2 * D:3 * D][cols]]).reshape(FT, 1)
        Wp = W_proj[g * HPC * HD:(g + 1) * HPC * HD, :]
        in_maps.append({
            "xT": xTb[b],
            "W1": np.ascontiguousarray(W1.astype(bf16)),
            "b1": np.ascontiguousarray(b1, dtype=np.float32),
            "Wp": np.ascontiguousarray(Wp, dtype=np.float32),
        })
    return in_maps


def kernel(x, W_qkv, b_qkv, W_proj, b_proj):
    from concourse.bass_utils import run_bass_kernel_spmd

    x = np.asarray(x, dtype=np.float32)
    W_qkv = np.asarray(W_qkv, dtype=np.float32)
    b_qkv = np.asarray(b_qkv, dtype=np.float32)
    W_proj = np.asarray(W_proj, dtype=np.float32)
    b_proj = np.asarray(b_proj, dtype=np.float32)

    if "nc" not in _CACHE:
        _CACHE["nc"] = _build()
    nc = _CACHE["nc"]

    in_maps = _shards(x, W_qkv, b_qkv, W_proj)
    res = run_bass_kernel_spmd(nc, in_maps, list(range(N_CORES)), trace=False)

    out = np.empty((B, S, D), dtype=np.float32)
    for b in range(B):
        acc = res.results[4 * b]["y"].astype(np.float32)
        for g in range(1, 4):
            acc = acc + res.results[4 * b + g]["y"].astype(np.float32)
        out[b] = acc + b_proj[None, :]
    return out


if __name__ == "__main__":
    rng = np.random.default_rng(0)
    scale = 1.0 / np.sqrt(D)
    inputs = {
        "x": rng.standard_normal((B, S, D), dtype=np.float32),
        "W_qkv": (rng.standard_normal((D, 3 * D)).astype(np.float32) * scale),
        "b_qkv": np.zeros(3 * D, np.float32),
        "W_proj": (rng.standard_normal((D, D)).astype(np.float32) * scale),
        "b_proj": np.zeros(D, np.float32),
    }
    out = kernel(**inputs)
    print("out", out.shape, out.dtype, np.abs(out).max())
